# revision 9
# baseline (speedup 1.0000x reference)
"""KNN edge-building kernel for Trainium2 (8 NeuronCores, SPMD).

Problem: B=4 graphs x M=8192 nodes, D=2 positions. Per graph: [M,M] squared
distances, top-k=16 smallest per row (self included, jax.lax.top_k stable
tie-breaking), emit edge list + distances normalized by the global max.

Sharding: 8 cores = (graph b, row-half h). Each core computes rows
[h*4096,(h+1)*4096) of graph b against all 8192 columns of that graph.

Device algorithm per 128-row tile (32 tiles/core):
  ACT : sq_x = Square(xj_rep - xi)   (per-partition bias = -xi)
  ACT : sq_y = Square(yj_rep - yi)
  POOL: nd2 = (sq_x * -1) - sq_y     (scalar_tensor_tensor, in-place)
  DVE : max8 -> top-8 of nd2 (= 8 smallest d2, descending nd2 order)
  DVE : max_index -> their indices (stable: ties get ascending indices)
  DVE : match_replace those 8 with -3e6 (in-place)
  DVE : max8 + max_index again -> ranks 9..16
Host: negate values, sqrt, normalize by global max, add per-graph index
offsets, assemble [2, N*k] edge index + pass-throughs.

The elementwise d2 formulation (fl(xj-xi)^2 + fl(yj-yi)^2) matches the
reference's rounding bit-for-bit, so the top-k ordering is exact.
"""

import sys
import numpy as np

B = 4
M = 8192
N = B * M
K = 16
D = 2
NCORES = 8
ROWS_PER_CORE = M * B // NCORES  # 4096
P = 128  # partitions / rows per tile
ZAP = -3.0e6  # below any real nd2 (nd2 in [-2e6, 0])


def _build_program(rows, cols):
    """Build the per-core Bass program. rows=4096, cols=8192 for the real
    problem; smaller for simulation tests."""
    import concourse.bass as bass
    import concourse.mybir as mybir
    from concourse import bacc
    from concourse import tile

    f32 = mybir.dt.float32
    u32 = mybir.dt.uint32
    AF = mybir.ActivationFunctionType
    ALU = mybir.AluOpType

    ntiles = rows // P
    nchunks = cols // 512

    nc = bacc.Bacc("TRN2", target_bir_lowering=False, debug=False)

    posr = nc.dram_tensor("posr", [rows, D], f32, kind="ExternalInput")
    posc = nc.dram_tensor("posc", [cols, D], f32, kind="ExternalInput")
    out_nvals = nc.dram_tensor("nvals", [rows, K], f32, kind="ExternalOutput")
    out_idx = nc.dram_tensor("idx", [rows, K], u32, kind="ExternalOutput")

    from concourse.masks import make_identity

    with tile.TileContext(nc) as tc:
        with (
            tc.tile_pool(name="rep", bufs=1) as rep_pool,
            tc.tile_pool(name="sqx", bufs=1) as sqx_pool,
            tc.tile_pool(name="sqy", bufs=1) as sqy_pool,
            tc.tile_pool(name="nd2", bufs=2) as nd2_pool,
            tc.tile_pool(name="small", bufs=3) as small_pool,
            tc.tile_pool(name="outs", bufs=3) as out_pool,
            tc.tile_pool(name="psum", bufs=8, space="PSUM") as psum_pool,
        ):
            # --- setup: replicate column coords to all 128 partitions ---
            # xy_rep[p, 2j] = xj, xy_rep[p, 2j+1] = yj for every partition p.
            # One broadcast DMA: 0-stride outer dim reads posc 128 times.
            xy_rep = rep_pool.tile([P, 2 * cols], f32)
            ident = rep_pool.tile([P, P], f32)
            make_identity(nc, ident[:])
            posc_flat = posc[:, :].rearrange("m d -> (m d)").unsqueeze(0)
            # split across partition ranges so multiple DMA queues run it
            PSPLIT = 8
            pstep = P // PSPLIT
            for s in range(PSPLIT):
                nc.sync.dma_start(
                    xy_rep[s * pstep : (s + 1) * pstep, :],
                    posc_flat.to_broadcast([pstep, 2 * cols]),
                )

            xj = xy_rep[:, 0 : 2 * cols : 2]
            yj = xy_rep[:, 1 : 2 * cols : 2]

            # --- main loop over row tiles ---
            for t in range(ntiles):
                xiyi = small_pool.tile([P, D], f32, tag="xiyi")
                nc.sync.dma_start(xiyi[:], posr[t * P : (t + 1) * P, :])

                sq_x = sqx_pool.tile([P, cols], f32, tag="sqx")
                sq_y = sqy_pool.tile([P, cols], f32, tag="sqy")
                # Square((-1)*xj + xi) == fl(xi-xj)^2, bit-identical to ref
                nc.scalar.activation(
                    sq_x[:], xj, AF.Square, bias=xiyi[:, 0:1], scale=-1.0
                )
                nc.scalar.activation(
                    sq_y[:], yj, AF.Square, bias=xiyi[:, 1:2], scale=-1.0
                )
                # nd2 = -(sq_x + sq_y): PE accumulates I@sq_x + I@sq_y into
                # PSUM (exact), ACT drains with scale=-1 (exact negate).
                nd2 = nd2_pool.tile([P, cols], f32, tag="nd2")
                for c in range(nchunks):
                    ps = psum_pool.tile([P, 512], f32, tag="ps")
                    sl = slice(c * 512, (c + 1) * 512)
                    nc.tensor.matmul(
                        ps[:], ident[:], sq_x[:, sl], start=True, stop=False
                    )
                    nc.tensor.matmul(
                        ps[:], ident[:], sq_y[:, sl], start=False, stop=True
                    )
                    nc.scalar.activation(
                        nd2[:, sl], ps[:], AF.Copy, bias=0.0, scale=-1.0
                    )

                nvals = out_pool.tile([P, K], f32, tag="nvals")
                idx = out_pool.tile([P, K], u32, tag="idx")
                nc.vector.max(nvals[:, 0:8], nd2[:])
                nc.vector.max_index(idx[:, 0:8], nvals[:, 0:8], nd2[:])
                nc.vector.match_replace(
                    out=nd2[:],
                    in_to_replace=nvals[:, 0:8],
                    in_values=nd2[:],
                    imm_value=ZAP,
                )
                nc.vector.max(nvals[:, 8:16], nd2[:])
                nc.vector.max_index(idx[:, 8:16], nvals[:, 8:16], nd2[:])

                nc.sync.dma_start(out_nvals[t * P : (t + 1) * P, :], nvals[:])
                nc.sync.dma_start(out_idx[t * P : (t + 1) * P, :], idx[:])

    nc.compile()
    return nc


def _run_device(posb, rows=ROWS_PER_CORE, cols=M):
    """posb: [B, M, D] f32. Returns (nvals [N,K] f32, idx [N,K] uint32)."""
    import os

    from concourse.bass_utils import run_bass_kernel_spmd

    nc = _build_program(rows, cols)
    halves = NCORES // B
    in_maps = []
    for d in range(NCORES):
        b, h = divmod(d, halves)
        in_maps.append(
            {
                "posr": np.ascontiguousarray(
                    posb[b, h * rows : (h + 1) * rows]
                ),
                "posc": np.ascontiguousarray(posb[b]),
            }
        )
    trace = bool(os.environ.get("KNN_TRACE"))
    if trace:
        try:
            sys.path.insert(0, "/root/problem")
            import ntff_shim

            ntff_shim.install()
        except Exception as e:
            print(f"ntff shim unavailable: {e}")
    r = run_bass_kernel_spmd(nc, in_maps, list(range(NCORES)), trace=trace)
    if trace:
        print(f"HW exec time: {r.exec_time_ns} ns")
        if r.instructions_and_trace is not None:
            print(f"trace path: {r.instructions_and_trace[1]}")
    res = r.results
    nvals = np.concatenate([r["nvals"] for r in res], axis=0)
    idx = np.concatenate([r["idx"] for r in res], axis=0)
    return nvals, idx


def kernel(x, pos, edge_index, edge_weight, batch, perm, score, i):
    x = np.asarray(x)
    pos = np.asarray(pos)
    perm_np = np.asarray(perm)

    pos_p = pos[perm_np]  # [N, D] pooled positions
    posb = np.ascontiguousarray(pos_p.reshape(B, M, D))

    nvals, idx = _run_device(posb)

    # values: nvals = top-16 of -d2 (descending) == d2 ascending
    d2k = -nvals
    dist = np.sqrt(np.maximum(d2k, np.float32(0.0)))
    new_edge_weight = (dist / dist.max()).reshape(-1)

    # indices: local column j -> global node id j + b*M
    idx_dtype = np.int32
    row_graph = np.repeat(np.arange(N, dtype=np.int64) // M, K).reshape(N, K)
    src = (idx.astype(np.int64) + row_graph * M).reshape(-1)
    tgt = np.repeat(np.arange(N, dtype=np.int64), K)
    new_edge_index = np.stack([src, tgt]).astype(idx_dtype)

    return (x, pos_p, new_edge_index, new_edge_weight, batch, perm, score)


# revision 10
# speedup vs baseline: 1.0376x; 1.0376x over previous
"""KNN edge-building kernel for Trainium2 (8 NeuronCores, SPMD).

Problem: B=4 graphs x M=8192 nodes, D=2 positions. Per graph: [M,M] squared
distances, top-k=16 smallest per row (self included, jax.lax.top_k stable
tie-breaking), emit edge list + distances normalized by the global max.

Sharding: 8 cores = (graph b, row-half h). Each core computes rows
[h*4096,(h+1)*4096) of graph b against all 8192 columns of that graph.

Device algorithm per 128-row tile (32 tiles/core):
  ACT : sq_x = Square(xj_rep - xi)   (per-partition bias = -xi)
  ACT : sq_y = Square(yj_rep - yi)
  POOL: nd2 = (sq_x * -1) - sq_y     (scalar_tensor_tensor, in-place)
  DVE : max8 -> top-8 of nd2 (= 8 smallest d2, descending nd2 order)
  DVE : max_index -> their indices (stable: ties get ascending indices)
  DVE : match_replace those 8 with -3e6 (in-place)
  DVE : max8 + max_index again -> ranks 9..16
Host: negate values, sqrt, normalize by global max, add per-graph index
offsets, assemble [2, N*k] edge index + pass-throughs.

The elementwise d2 formulation (fl(xj-xi)^2 + fl(yj-yi)^2) matches the
reference's rounding bit-for-bit, so the top-k ordering is exact.
"""

import sys
import numpy as np

B = 4
M = 8192
N = B * M
K = 16
D = 2
NCORES = 8
ROWS_PER_CORE = M * B // NCORES  # 4096
P = 128  # partitions / rows per tile
ZAP = -3.0e6  # below any real nd2 (nd2 in [-2e6, 0])


def _build_program(rows, cols):
    """Build the per-core Bass program. rows=4096, cols=8192 for the real
    problem; smaller for simulation tests."""
    import concourse.bass as bass
    import concourse.mybir as mybir
    from concourse import bacc
    from concourse import tile

    f32 = mybir.dt.float32
    u32 = mybir.dt.uint32
    AF = mybir.ActivationFunctionType
    ALU = mybir.AluOpType

    ntiles = rows // P
    nchunks = cols // 512

    nc = bacc.Bacc("TRN2", target_bir_lowering=False, debug=False)

    posr = nc.dram_tensor("posr", [rows, D], f32, kind="ExternalInput")
    posc = nc.dram_tensor("posc", [cols, D], f32, kind="ExternalInput")
    out_nvals = nc.dram_tensor("nvals", [rows, K], f32, kind="ExternalOutput")
    out_idx = nc.dram_tensor("idx", [rows, K], u32, kind="ExternalOutput")

    from concourse.masks import make_identity

    with tile.TileContext(nc) as tc:
        with (
            tc.tile_pool(name="rep", bufs=1) as rep_pool,
            tc.tile_pool(name="sqx", bufs=1) as sqx_pool,
            tc.tile_pool(name="sqy", bufs=1) as sqy_pool,
            tc.tile_pool(name="nd2", bufs=2) as nd2_pool,
            tc.tile_pool(name="small", bufs=3) as small_pool,
            tc.tile_pool(name="outs", bufs=3) as out_pool,
            tc.tile_pool(name="psum", bufs=8, space="PSUM") as psum_pool,
        ):
            # --- setup: replicate column coords to all 128 partitions ---
            # xy_rep[p, 2j] = xj, xy_rep[p, 2j+1] = yj for every partition p.
            # One broadcast DMA: 0-stride outer dim reads posc 128 times.
            xy_rep = rep_pool.tile([P, 2 * cols], f32)
            ident = rep_pool.tile([P, P], f32)
            make_identity(nc, ident[:])
            posc_bcast = (
                posc[:, :]
                .rearrange("m d -> (m d)")
                .unsqueeze(0)
                .to_broadcast([P, 2 * cols])
            )
            nc.sync.dma_start(xy_rep[:], posc_bcast)

            xj = xy_rep[:, 0 : 2 * cols : 2]
            yj = xy_rep[:, 1 : 2 * cols : 2]

            # --- main loop over row tiles ---
            for t in range(ntiles):
                xiyi = small_pool.tile([P, D], f32, tag="xiyi")
                nc.sync.dma_start(xiyi[:], posr[t * P : (t + 1) * P, :])

                sq_x = sqx_pool.tile([P, cols], f32, tag="sqx")
                sq_y = sqy_pool.tile([P, cols], f32, tag="sqy")
                # Square((-1)*xj + xi) == fl(xi-xj)^2, bit-identical to ref
                nc.scalar.activation(
                    sq_x[:], xj, AF.Square, bias=xiyi[:, 0:1], scale=-1.0
                )
                nc.scalar.activation(
                    sq_y[:], yj, AF.Square, bias=xiyi[:, 1:2], scale=-1.0
                )
                # nd2 = -(sq_x + sq_y): PE accumulates I@sq_x + I@sq_y into
                # PSUM (exact), ACT drains with scale=-1 (exact negate).
                nd2 = nd2_pool.tile([P, cols], f32, tag="nd2")
                for c in range(nchunks):
                    ps = psum_pool.tile([P, 512], f32, tag="ps")
                    sl = slice(c * 512, (c + 1) * 512)
                    nc.tensor.matmul(
                        ps[:], ident[:], sq_x[:, sl], start=True, stop=False
                    )
                    nc.tensor.matmul(
                        ps[:], ident[:], sq_y[:, sl], start=False, stop=True
                    )
                    nc.scalar.activation(
                        nd2[:, sl], ps[:], AF.Copy, bias=0.0, scale=-1.0
                    )

                nvals = out_pool.tile([P, K], f32, tag="nvals")
                idx = out_pool.tile([P, K], u32, tag="idx")
                nc.vector.max(nvals[:, 0:8], nd2[:])
                nc.vector.max_index(idx[:, 0:8], nvals[:, 0:8], nd2[:])
                nc.vector.match_replace(
                    out=nd2[:],
                    in_to_replace=nvals[:, 0:8],
                    in_values=nd2[:],
                    imm_value=ZAP,
                )
                nc.vector.max(nvals[:, 8:16], nd2[:])
                nc.vector.max_index(idx[:, 8:16], nvals[:, 8:16], nd2[:])

                nc.sync.dma_start(out_nvals[t * P : (t + 1) * P, :], nvals[:])
                nc.sync.dma_start(out_idx[t * P : (t + 1) * P, :], idx[:])

    nc.compile()
    return nc


def _run_device(posb, rows=ROWS_PER_CORE, cols=M):
    """posb: [B, M, D] f32. Returns (nvals [N,K] f32, idx [N,K] uint32)."""
    import os

    from concourse.bass_utils import run_bass_kernel_spmd

    nc = _build_program(rows, cols)
    halves = NCORES // B
    in_maps = []
    for d in range(NCORES):
        b, h = divmod(d, halves)
        in_maps.append(
            {
                "posr": np.ascontiguousarray(
                    posb[b, h * rows : (h + 1) * rows]
                ),
                "posc": np.ascontiguousarray(posb[b]),
            }
        )
    trace = bool(os.environ.get("KNN_TRACE"))
    if trace:
        try:
            sys.path.insert(0, "/root/problem")
            import ntff_shim

            ntff_shim.install()
        except Exception as e:
            print(f"ntff shim unavailable: {e}")
    r = run_bass_kernel_spmd(nc, in_maps, list(range(NCORES)), trace=trace)
    if trace:
        print(f"HW exec time: {r.exec_time_ns} ns")
        if r.instructions_and_trace is not None:
            print(f"trace path: {r.instructions_and_trace[1]}")
    res = r.results
    nvals = np.concatenate([r["nvals"] for r in res], axis=0)
    idx = np.concatenate([r["idx"] for r in res], axis=0)
    return nvals, idx


def kernel(x, pos, edge_index, edge_weight, batch, perm, score, i):
    x = np.asarray(x)
    pos = np.asarray(pos)
    perm_np = np.asarray(perm)

    pos_p = pos[perm_np]  # [N, D] pooled positions
    posb = np.ascontiguousarray(pos_p.reshape(B, M, D))

    nvals, idx = _run_device(posb)

    # values: nvals = top-16 of -d2 (descending) == d2 ascending
    d2k = -nvals
    dist = np.sqrt(np.maximum(d2k, np.float32(0.0)))
    new_edge_weight = (dist / dist.max()).reshape(-1)

    # indices: local column j -> global node id j + b*M
    idx_dtype = np.int32
    row_graph = np.repeat(np.arange(N, dtype=np.int64) // M, K).reshape(N, K)
    src = (idx.astype(np.int64) + row_graph * M).reshape(-1)
    tgt = np.repeat(np.arange(N, dtype=np.int64), K)
    new_edge_index = np.stack([src, tgt]).astype(idx_dtype)

    return (x, pos_p, new_edge_index, new_edge_weight, batch, perm, score)
